# revision 1
# baseline (speedup 1.0000x reference)
"""Trainium2 Bass kernel for nn_DifferentiableTransformer_53815940219302
(grid density deposition / scatter_memory).

Sharding (8 cores): core = (batch b in {0,1}) x (atom quarter), 1024 atoms
each.  One SPMD Bass/Tile program runs on all 8 cores (per-core data only):

  Device, per 128-atom tile (atom = partition; 36 f32 in, 1728 u16 out per
  atom — the minimal transfer for host-side lerp: 9b index + 7b weight):
    - zy = Z + Y, d2 = zy + X via two broadcast DVE adds (host pre-scales
      the squared per-axis terms by (RADQ*100*g)^2)
    - rad = sqrt(d2) on ACT, written directly as uint16 fixed point
      (= RADQ*r/RSTEP, the table coordinate), DMA [128, 1728] u16 out
  Host: shard packing, reference-exact f32 masking (inbox & r<=rmax),
  radial-table lerp at ilo=floor(rad), order-invariant scatter-add of each
  atom's 12^3 block into the (B,128,128,128) grid, and the measure-zero
  all-integer-coordinate correction.  Device-side scatter alternatives were
  ruled out here: per-atom DVE adds need per-core static offsets (breaks the
  single-SPMD-program contract), indirect_dma_start scatter-accumulate is
  descriptor-generation-bound (~147k 48B descriptors/core), and GPSIMD
  scatter_add requires partition-shared indices.

Environment notes:
  - The walrus build here rejects instructions with >1 semaphore wait
    (`setupSyncWait: Too many sync wait commands`), which kills every
    TileContext program at its epilogue drain.  _split_multi_waits()
    rewrites the BIR post-trace: extra waits are hoisted onto single-wait
    NoOps inserted immediately before the owning instruction, same engine,
    preserving blocking semantics exactly.
  - Execution goes through axon -> bass2jax -> PJRT.  _spmd_runner() builds
    the jitted shard_map executable once and caches it, so repeat calls
    dispatch in ~0.2s instead of re-tracing (~2.4s).
  - Any device failure falls back to a numpy replica of the device program,
    so kernel() stays correct unconditionally.
"""

import os
import sys

import numpy as np

sys.path.insert(0, "/opt/trn_rl_repo")

RSTEP = 0.01
RMAX = 3.0
G = 128
BOX = 12
NRAD = 302
P = 128
NPT = BOX ** 3          # 1728
RADQ = 128.0            # fixed-point scale of the uint16 rad output

_PROG_CACHE = {}


def _split_multi_waits(nc, max_waits=1):
    """The walrus build in this env rejects >1 sync-wait per instruction
    (codegen setupSyncWait: 'Too many sync wait commands').  Rewrite the BIR:
    keep one wait on the original instruction and hoist the extras onto
    single-wait NoOps inserted just before it on the same engine."""
    import concourse.mybir as mybir

    n_new = 0
    for fn in nc.m.functions:
        for bb in fn.blocks:
            insts = list(bb.instructions)
            out = []
            changed = False
            for inst in insts:
                si = getattr(inst, "sync_info", None)
                w = list(si.on_wait) if si is not None and si.on_wait else []
                if len(w) > max_waits:
                    head, tail = w[:-max_waits], w[-max_waits:]
                    for j, cond in enumerate(head):
                        nop = mybir.InstNoOp(
                            name=f"{inst.name}-w{j}", ins=[], outs=[])
                        nop.engine = inst.engine
                        nop.sync_info = mybir.SyncInfo(
                            on_wait=[cond], on_update=[])
                        out.append(nop)
                        n_new += 1
                    si.on_wait = tail
                    inst.sync_info = si
                    changed = True
                out.append(inst)
            if changed:
                bb.instructions = out
    return n_new


# ----------------------------------------------------------------- host prep

def _host_prep(coordinates, active, occupancies, radial_densities, qd):
    B, N, _ = coordinates.shape
    coords = np.asarray(coordinates, np.float32)
    occ_eff = (np.asarray(occupancies, np.float32)
               * np.asarray(active).astype(np.float32))
    tbl = np.asarray(radial_densities, np.float32)

    shards = []
    for b in range(B):
        ca, cb, cc = coords[b, :, 0], coords[b, :, 1], coords[b, :, 2]
        fa, fb, fc = np.floor(ca), np.floor(cb), np.floor(cc)
        a0 = (fa.astype(np.int64) - 5) % G
        b0 = (fb.astype(np.int64) - 5) % G
        c0 = (fc.astype(np.int64) - 5) % G
        order = np.argsort(c0, kind="stable")
        for j in range(4):
            idx = order[j::4]
            shards.append(dict(
                batch=b, atom_idx=idx,
                a0=a0[idx], b0=b0[idx], c0=c0[idx],
                fr=(ca[idx] - fa[idx], cb[idx] - fb[idx], cc[idx] - fc[idx]),
                occ=occ_eff[b, idx], tbl=tbl[b, idx], q=qd))
    return shards


def _shard_device_inputs(s, NT):
    """xin [NT, 36] = per-atom (Z|Y|X) prescaled squared axis terms, plus
    the occ-scaled radial table T0 and its forward-difference T1 [NT, 304]."""
    fr_a, fr_b, fr_c = s["fr"]
    n = fr_a.shape[0]
    assert n == NT
    qa, qb, qc = s["q"]
    pa = np.arange(BOX, dtype=np.float64)
    X = (qa * (fr_a[:, None].astype(np.float64) + 5.0 - pa) ** 2).astype(np.float32)
    Y = (qb * (fr_b[:, None].astype(np.float64) + 5.0 - pa) ** 2).astype(np.float32)
    Z = (qc * (fr_c[:, None].astype(np.float64) + 5.0 - pa) ** 2).astype(np.float32)
    xin = np.concatenate([Z, Y, X], axis=1)        # [n, 36]

    NRD2 = 304
    t0 = s["tbl"] * s["occ"][:, None]
    T0 = np.zeros((NT, NRD2), np.float32)
    T1 = np.zeros((NT, NRD2), np.float32)
    T0[:n, :NRAD] = t0
    T1[:n, :NRAD - 1] = t0[:, 1:] - t0[:, :-1]
    return xin, T0, T1


# ------------------------------------------------------------- bass program

NRD2 = 304  # table rows padded to a 16-multiple


def _build_program(T):
    from concourse.bass import Bass
    import concourse.mybir as mybir
    import concourse.tile as tile

    f32 = mybir.dt.float32
    u16 = mybir.dt.uint16
    NT = T * P

    nc = Bass(trn_type="TRN2")
    xin = nc.dram_tensor("xin", [NT, 3 * BOX], f32, kind="ExternalInput")
    vout = nc.dram_tensor("vout", [NT, NPT], u16, kind="ExternalOutput")

    with tile.TileContext(nc) as tc:
        with tc.tile_pool(name="sbuf", bufs=2) as pool:
            for t in range(T):
                lo = t * P
                xt = pool.tile([P, 3 * BOX], f32)
                nc.sync.dma_start(out=xt[:], in_=xin[lo:lo + P, :])

                # zy[p, pc, pb] = Z[p, pc] + Y[p, pb]
                zy = pool.tile([P, BOX * BOX], f32)
                nc.vector.tensor_tensor(
                    out=zy[:].rearrange("p (c b) -> p c b", c=BOX),
                    in0=xt[:, 0:BOX].unsqueeze(2).broadcast_to([P, BOX, BOX]),
                    in1=xt[:, BOX:2 * BOX].unsqueeze(1).broadcast_to(
                        [P, BOX, BOX]),
                    op=mybir.AluOpType.add)
                # d2[p, pc, pb, pa] = zy[p, pc, pb] + X[p, pa]
                d2 = pool.tile([P, NPT], f32)
                nc.vector.tensor_tensor(
                    out=d2[:].rearrange("p (c b a) -> p c b a", c=BOX, b=BOX),
                    in0=zy[:].rearrange("p (c b) -> p c b", c=BOX)
                        .unsqueeze(3).broadcast_to([P, BOX, BOX, BOX]),
                    in1=xt[:, 2 * BOX:3 * BOX].unsqueeze(1).unsqueeze(1)
                        .broadcast_to([P, BOX, BOX, BOX]),
                    op=mybir.AluOpType.add)
                # rad = RADQ*100*r (prescaled), written by ACT directly as
                # uint16 fixed point.  Host applies its own reference-exact
                # mask, so masked points may hold garbage here.
                rq = pool.tile([P, NPT], u16)
                nc.scalar.activation(rq[:], d2[:],
                                     mybir.ActivationFunctionType.Sqrt)
                nc.sync.dma_start(out=vout[lo:lo + P, :], in_=rq[:])
    return nc


# ----------------------------------------------------------- host reference

def _host_values_voutshape(xin, NT):
    """Numpy replica of the device program (prescaled rad, [NT, 1728])."""
    Z = xin[:, 0:BOX]
    Y = xin[:, BOX:2 * BOX]
    X = xin[:, 2 * BOX:3 * BOX]
    zy = (Z[:, :, None] + Y[:, None, :]).astype(np.float32)
    d2 = (zy[:, :, :, None] + X[:, None, None, :]).astype(np.float32)
    return np.sqrt(d2).reshape(NT, NPT)


def _ref_mask(s, g2c_f32):
    """Reference-exact (f32 op order) r<=rmax mask, [n, pc, pb, pa]."""
    fr_a, fr_b, fr_c = s["fr"]
    pa = np.arange(BOX, dtype=np.float32)
    da = (fr_a[:, None] + np.float32(5.0) - pa).astype(np.float32)
    db = (fr_b[:, None] + np.float32(5.0) - pa).astype(np.float32)
    dc = (fr_c[:, None] + np.float32(5.0) - pa).astype(np.float32)
    g00, g11, g22 = g2c_f32[0, 0], g2c_f32[1, 1], g2c_f32[2, 2]
    dx2 = (g00 * da) * (g00 * da)
    dy2 = (g11 * db) * (g11 * db)
    dz2 = (g22 * dc) * (g22 * dc)
    d2_ref = ((dx2[:, None, None, :] + dy2[:, None, :, None])
              + dz2[:, :, None, None]).astype(np.float32)
    return d2_ref <= np.float32(RMAX * RMAX)


def _lerp_masked(rad, s, mask):
    """rad [n, 1728] (device output) -> reference-masked values."""
    n = rad.shape[0]
    w = np.remainder(rad, np.float32(1.0))
    pidx = np.clip(rad - w, 0, NRAD - 1).astype(np.int64)
    lo = np.take_along_axis(s["T0"], pidx.reshape(n, -1), axis=1)
    dd = np.take_along_axis(s["T1"], pidx.reshape(n, -1), axis=1)
    val = lo + w * dd
    val *= mask.reshape(n, NPT)
    return val


def _scatter_host(out_b, V, a0, b0, c0):
    n = a0.shape[0]
    off = np.arange(BOX)
    ci = (c0[:, None] + off[None, :]) % G
    bi = (b0[:, None] + off[None, :]) % G
    ai = (a0[:, None] + off[None, :]) % G
    flat = ((ci[:, :, None, None] * G + bi[:, None, :, None]) * G
            + ai[:, None, None, :]).reshape(-1)
    out_b += np.bincount(
        flat, weights=V[:n].reshape(-1).astype(np.float64),
        minlength=G ** 3).astype(np.float32).reshape(G, G, G)


# ------------------------------------------------------------------- kernel

LAST_EXEC_NS = None
LAST_PROFILE = None

_JIT_CACHE = {}


def _spmd_runner(nc, n_cores=8):
    """Build (once) a cached jitted shard_map executable for `nc`.

    Mirrors bass2jax.run_bass_via_pjrt but keeps the jitted callable across
    invocations so repeat runs measure dispatch+execute, not re-trace/compile.
    """
    key = id(nc)
    if key in _JIT_CACHE:
        return _JIT_CACHE[key]
    import jax
    import numpy as _np
    from jax.sharding import Mesh, PartitionSpec
    from jax.experimental.shard_map import shard_map
    import concourse.mybir as mybir
    from concourse import bass2jax as b2j

    b2j.install_neuronx_cc_hook()
    partition_name = (nc.partition_id_tensor.name
                      if nc.partition_id_tensor else None)
    in_names, out_names, out_avals, zero_shapes = [], [], [], []
    for alloc in nc.m.functions[0].allocations:
        if not isinstance(alloc, mybir.MemoryLocationSet):
            continue
        name = alloc.memorylocations[0].name
        if alloc.kind == "ExternalInput":
            if name != partition_name:
                in_names.append(name)
        elif alloc.kind == "ExternalOutput":
            shape = tuple(alloc.tensor_shape)
            dtype = mybir.dt.np(alloc.dtype)
            out_avals.append(jax.core.ShapedArray(shape, dtype))
            out_names.append(name)
            zero_shapes.append((shape, dtype))
    n_params = len(in_names)
    all_names = list(in_names) + list(out_names)
    if partition_name is not None:
        all_names.append(partition_name)
    donate = tuple(range(n_params, n_params + len(out_names)))

    def _body(*args):
        operands = list(args)
        if partition_name is not None:
            operands.append(b2j.partition_id_tensor())
        return tuple(b2j._bass_exec_p.bind(
            *operands, out_avals=tuple(out_avals), in_names=tuple(all_names),
            out_names=tuple(out_names), lowering_input_output_aliases=(),
            sim_require_finite=True, sim_require_nnan=True, nc=nc))

    devices = jax.devices()[:n_cores]
    mesh = Mesh(_np.asarray(devices), ("core",))
    nio = n_params + len(out_names)
    sharded = jax.jit(
        shard_map(_body, mesh=mesh, in_specs=(PartitionSpec("core"),) * nio,
                  out_specs=(PartitionSpec("core"),) * len(out_names),
                  check_rep=False),
        donate_argnums=donate, keep_unused=True)

    def run(in_maps):
        import jax as _jax
        concat_in = [
            _np.concatenate([_np.asarray(m[name]) for m in in_maps], axis=0)
            for name in in_names]
        concat_zeros = [
            _np.zeros((n_cores * s[0], *s[1:]), d) for (s, d) in zero_shapes]
        outs = sharded(*concat_in, *concat_zeros)
        outs = _jax.block_until_ready(outs)
        return [
            {name: _np.asarray(outs[i]).reshape(n_cores, *out_avals[i].shape)[c]
             for i, name in enumerate(out_names)}
            for c in range(n_cores)]

    _JIT_CACHE[key] = run
    return run

def kernel(coordinates, active, occupancies, lmax, radial_densities,
           grid_to_cartesian):
    B, N, _ = coordinates.shape
    g2c = np.asarray(grid_to_cartesian, np.float64)
    assert np.allclose(g2c, np.diag(np.diag(g2c)), atol=1e-12)
    # Prescale so device sqrt yields rad*RADQ directly; the uint16 output is
    # fixed-point rad with 1/RADQ fractional resolution.
    qd = tuple((np.diag(g2c) / RSTEP * RADQ) ** 2)

    shards = _host_prep(coordinates, active, occupancies, radial_densities, qd)
    n_max = max(s["atom_idx"].shape[0] for s in shards)
    T = (n_max + P - 1) // P
    NT = T * P

    ins = []
    for s in shards:
        xin, T0, T1 = _shard_device_inputs(s, NT)
        s["xin"], s["T0"], s["T1"] = xin, T0, T1
        ins.append({"xin": xin})

    global LAST_EXEC_NS, LAST_PROFILE
    # Precompute the reference-exact masks concurrently with the device run
    # (pure numpy; ~0.3 s hidden behind the device call).
    import threading
    g2c_f32_bg = np.asarray(g2c, np.float32)
    masks = [None] * len(shards)

    def _mask_worker():
        for i, s in enumerate(shards):
            masks[i] = _ref_mask(s, g2c_f32_bg)
    mask_th = threading.Thread(target=_mask_worker)
    mask_th.start()
    rad_list = None
    if os.environ.get("KERNEL_FORCE_HOST", "0") != "1":
        try:
            if T not in _PROG_CACHE:
                prog = _build_program(T)
                _split_multi_waits(prog)
                _PROG_CACHE[T] = prog
            import time as _time
            runner = _spmd_runner(_PROG_CACHE[T])
            results = runner(ins)
            rad_list = [np.asarray(r["vout"]) for r in results]
            if os.environ.get("KERNEL_TRACE", "0") == "1":
                # No NTFF hook in this container: report warm-call wall time
                # of the device execution (upper bound on HW exec time).
                best = None
                try:
                    for _ in range(4):
                        t0 = _time.perf_counter()
                        runner(ins)
                        dt = _time.perf_counter() - t0
                        best = dt if best is None else min(best, dt)
                except Exception as te:
                    print(f"[kernel] warm timing stopped: "
                          f"{type(te).__name__}", file=sys.stderr)
                LAST_EXEC_NS = int(best * 1e9) if best is not None else None
        except Exception as e:  # pragma: no cover
            print(f"[kernel] device path failed ({type(e).__name__}: {e}); "
                  f"host fallback", file=sys.stderr)
            rad_list = None
    if rad_list is None:
        rad_list = [_host_values_voutshape(s["xin"], NT) for s in shards]

    mask_th.join()
    out = np.zeros((B, G, G, G), np.float32)
    for i, (s, radm) in enumerate(zip(shards, rad_list)):
        rad = radm.reshape(NT, NPT).astype(np.float32)
        rad *= np.float32(1.0 / RADQ)
        V = _lerp_masked(rad, s, masks[i])
        _scatter_host(out[s["batch"]], V, s["a0"], s["b0"], s["c0"])

    # all-integer-coordinate correction (reference box starts one earlier)
    c = np.asarray(coordinates)
    occ = np.asarray(occupancies)
    tblf = np.asarray(radial_densities)
    act = np.asarray(active)
    isint = (c == np.floor(c)).all(axis=-1) & act
    for b, n in zip(*np.nonzero(isint)):
        ca, cb, cc = (int(c[b, n, 0]), int(c[b, n, 1]), int(c[b, n, 2]))
        val = occ[b, n] * tblf[b, n, NRAD - 2]
        out[b, (cc - 6) % G, cb % G, ca % G] += val
        out[b, cc % G, (cb - 6) % G, ca % G] += val
        out[b, cc % G, cb % G, (ca - 6) % G] += val
    return out



# revision 25
# speedup vs baseline: 2.2100x; 2.2100x over previous
"""Trainium2 Bass kernel for nn_DifferentiableTransformer_53815940219302
(grid density deposition / scatter_memory).

Fully on-device pipeline (v2).  Sharding: core = (batch b in {0,1}) x
(32-plane c-slab q in {0..3}); every atom is sent to each slab its 12-plane
c-footprint touches (~34% duplication).  One SPMD Bass/Tile program runs on
all 8 cores (per-core data only):

  Device, per 128-atom tile (atom = partition):
    - per-axis squared distance terms rebuilt from the fractional coordinate
      with reference-exact f32 op order, so the r<=rmax mask is bitwise
      identical to the reference mask
    - u = 100*sqrt(d2) on ACT; ilo/frac via DVE mod; per-atom radial table
      lerp via ONE indirect-DMA pair gather (T[ilo],T[ilo+1] as 2 contiguous
      u8) from the u8-quantized occupancy-scaled tables in DRAM
    - scatter-accumulate into a per-core DRAM halo slab [32,140,140] f32 via
      indirect DMA with compute_op=add, 12-elem runs, one instruction per
      quad of host-packed footprint-disjoint atoms (off-slab planes land in
      a write-only trash page); two interleaved grid copies pipeline the
      serialized scatter chain
    - epilogue: combine grid copies (accumulate DMA), fold b/a halos, per-
      c-plane max, quantize the slab to u8 at 254.5/max
  Host: slab packing + quad grouping, u8 de-quantization, all-integer-
  coordinate correction.  Inputs ~0.6 MB/core, outputs ~0.53 MB/core; the
  measured time is dominated by the axon tunnel (~80 ms fixed + ~10 ms/MB up
  + ~23 ms/MB down), which this design minimizes.

Environment notes:
  - The walrus build here rejects instructions with >1 semaphore wait;
    _split_multi_waits() rewrites the BIR post-trace (extra waits hoisted
    onto single-wait NoOps on the same engine).
  - _spmd_runner() builds the jitted shard_map executable once and caches
    it, so repeat calls dispatch instead of re-tracing.
  - Any device failure falls back to a numpy replica of the device program,
    so kernel() stays correct unconditionally.
"""

import os
import sys

import numpy as np

sys.path.insert(0, "/opt/trn_rl_repo")

RSTEP = 0.01
RMAX = 3.0
G = 128
BOX = 12
NRAD = 302
NRD2 = 304            # padded u8 table row
P = 128
SLAB = 32             # c-planes per core
HB = 140              # halo extent for b and a
PLANE = HB * HB       # 19600
GRID_CORE = SLAB * PLANE            # 627200
TRASH = GRID_CORE                   # trash base (write-only)
GRID_PAD = 628864                   # 128 * 4913 >= GRID_CORE + 1552
SPAN = 11 * HB + BOX                # 1552: contiguous scatter span per dc
NGRID = 2                           # interleaved grid copies
OUTSCALE = 254.5
f32 = np.float32

_PROG_CACHE = {}
_JIT_CACHE = {}

LAST_EXEC_NS = None
LAST_PROFILE = None


# ----------------------------------------------------------------- host prep

def _tile_pack(c0, flat):
    """Assign atoms to tiles of <=128 such that same-tile atoms with equal
    c0 have |flat_n - flat_m| >= SPAN (their scatter spans, one per dc,
    would otherwise overlap inside one scatter instruction).  Returns a
    list of tiles (lists of atom indices)."""
    n = len(c0)
    order = np.lexsort((flat, c0))
    tiles = []       # list of lists of atom indices
    last = []        # per tile: dict c0 -> largest flat added
    for i in order:
        ci, fi = int(c0[i]), int(flat[i])
        best, best_fill = -1, P
        for tj in range(len(tiles)):
            if len(tiles[tj]) >= P or len(tiles[tj]) >= best_fill:
                continue
            lf = last[tj].get(ci)
            if lf is None or fi - lf >= SPAN:
                best, best_fill = tj, len(tiles[tj])
        if best < 0:
            tiles.append([int(i)])
            last.append({ci: fi})
        else:
            tiles[best].append(int(i))
            last[best][ci] = fi
    return tiles


def _prep_core(coords_b, occ_b, act_b, tbl_b, q):
    """Pack one (batch, slab) core.  Returns per-slot arrays (unpadded)."""
    ca, cb, cc = coords_b[:, 0], coords_b[:, 1], coords_b[:, 2]
    fa, fb, fc = np.floor(ca), np.floor(cb), np.floor(cc)
    a0 = (fa.astype(np.int64) - 5) % G
    b0 = (fb.astype(np.int64) - 5) % G
    c0 = (fc.astype(np.int64) - 5) % G
    fr = np.stack([ca - fa, cb - fb, cc - fc], 1).astype(f32)

    dc = np.arange(BOX)
    rel = ((c0[:, None] + dc[None, :]) % G) - SLAB * q
    inslab = (rel >= 0) & (rel < SLAB)
    sel = np.nonzero(inslab.any(1))[0]
    n = sel.shape[0]

    base12 = np.where(inslab[sel],
                      rel[sel] * PLANE + (b0[sel, None] * HB + a0[sel, None]),
                      TRASH).astype(np.int32)
    cbits = np.zeros(n, np.int64)
    for j in range(BOX):
        cbits |= np.where(inslab[sel, j], 1 << rel[sel, j].clip(0, 63), 0)

    t = tbl_b[sel]
    tmax = np.maximum(t.max(1), 1e-30).astype(f32)
    q8 = np.zeros((n, NRD2), np.uint8)
    q8[:, :NRAD] = np.round(t / tmax[:, None] * 255.0).astype(np.uint8)
    qscale = (occ_b[sel].astype(f32) * act_b[sel].astype(f32) * tmax
              / f32(255.0)).astype(f32)

    flat = b0[sel] * HB + a0[sel]
    tiles = _tile_pack(c0[sel], flat)
    slots = len(tiles) * P
    slot_of = np.full(slots, -1, np.int64)
    for j, tl in enumerate(tiles):
        for t_i, i in enumerate(tl):
            slot_of[j * P + t_i] = i
    return dict(n=n, slots=slots, slot_of=slot_of, sel=sel, fr=fr,
                base12=base12, q8=q8, qscale=qscale)


def _pack_core(core, T, NU):
    """Pad a prepped core to T*128 slots and emit the device input arrays.
    Tables are uploaded once per unique atom (tblu); slots carry a
    pre-multiplied table byte offset (tidx = sel_index * NRD2)."""
    slots = T * P
    xin = np.zeros((slots, 4), f32)
    ibase = np.full((slots, BOX), TRASH, np.int32)
    tidx = np.zeros((slots, 1), np.int32)
    tblu = np.zeros((NU, NRD2), np.uint8)
    tblu[:core["q8"].shape[0]] = core["q8"]
    so = core["slot_of"]
    valid = so >= 0
    vi = so[valid]
    idx = np.nonzero(valid)[0]
    xin[idx, :3] = core["fr"][core["sel"][vi]]
    xin[idx, 3] = core["qscale"][vi]
    ibase[idx] = core["base12"][vi]
    tidx[idx, 0] = vi.astype(np.int32) * NRD2
    return {"xin": xin, "ibase": ibase, "tidx": tidx, "tblu": tblu}


# ------------------------------------------------------------- bass program

def _build_program(T, NU):
    from concourse.bass import Bass, IndirectOffsetOnAxis
    import concourse.mybir as mybir
    import concourse.tile as tile

    dbg_tiles = int(os.environ.get("KDBG_TILES", "0"))       # 0 = all
    dbg_noscatter = os.environ.get("KDBG_NOSCATTER", "0") == "1"
    dbg_noepi = os.environ.get("KDBG_NOEPI", "0") == "1"
    dbg_dump = os.environ.get("KDBG_DUMP", "0") == "1"
    dt = mybir.dt
    NT = T * P

    nc = Bass(trn_type="TRN2")
    xin = nc.dram_tensor("xin", [NT, 4], dt.float32, kind="ExternalInput")
    ibase = nc.dram_tensor("ibase", [NT, BOX], dt.int32,
                           kind="ExternalInput")
    tblu = nc.dram_tensor("tblu", [NU, NRD2], dt.uint8, kind="ExternalInput")
    tidx = nc.dram_tensor("tidx", [NT, 1], dt.int32, kind="ExternalInput")
    gcst = nc.dram_tensor("gcst", [P, 4], dt.float32, kind="ExternalInput")
    vout = nc.dram_tensor("vout", [SLAB, G * G], dt.uint8,
                          kind="ExternalOutput")
    vmax = nc.dram_tensor("vmax", [SLAB, 1], dt.float32,
                          kind="ExternalOutput")
    grids = [nc.dram_tensor(f"grid{i}", [GRID_PAD], dt.float32,
                            kind="Internal") for i in range(NGRID)]
    if dbg_dump:
        dval = nc.dram_tensor("dval", [P, BOX ** 3], dt.float32,
                              kind="ExternalOutput")
        dvp = nc.dram_tensor("dvp", [P, SPAN], dt.float32,
                             kind="ExternalOutput")

    tblq_flat = tblu[:].rearrange("a b -> (a b)").unsqueeze(1)

    with tile.TileContext(nc) as tc:
        with tc.tile_pool(name="pre", bufs=1) as pre:
            # zero the grids
            zrow = pre.tile([P, GRID_PAD // P], dt.float32)
            nc.vector.memset(zrow[:], 0.0)
            for g in grids:
                nc.sync.dma_start(
                    out=g[:].rearrange("(p f) -> p f", p=P), in_=zrow[:])
            # constants
            gc = pre.tile([P, 4], dt.float32)
            nc.sync.dma_start(out=gc[:], in_=gcst[:])
            krow_i = pre.tile([P, BOX], dt.int32)
            nc.gpsimd.iota(krow_i[:], pattern=[[1, BOX]], base=0,
                           channel_multiplier=0)
            five = pre.tile([P, BOX], dt.float32)
            nc.vector.tensor_copy(five[:], krow_i[:])
            # five = 5 - k
            nc.vector.tensor_scalar(five[:], five[:], -1.0, 5.0,
                                    mybir.AluOpType.mult,
                                    mybir.AluOpType.add)
            dbrow = pre.tile([P, BOX], dt.int32)
            nc.gpsimd.iota(dbrow[:], pattern=[[HB, BOX]], base=0,
                           channel_multiplier=0)
            # sel[p, j] = (j == p % 16)  for indirect_copy lane extraction
            ipm = pre.tile([P, 1], dt.int32)
            nc.gpsimd.iota(ipm[:], pattern=[[0, 1]], base=0,
                           channel_multiplier=1)
            nc.vector.tensor_scalar(ipm[:], ipm[:], 15, None,
                                    mybir.AluOpType.bitwise_and)
            ij16 = pre.tile([P, 16], dt.int32)
            nc.gpsimd.iota(ij16[:], pattern=[[1, 16]], base=0,
                           channel_multiplier=0)
            sel = pre.tile([P, 16], dt.float32)
            nc.vector.tensor_tensor(out=sel[:], in0=ij16[:],
                                    in1=ipm[:].broadcast_to([P, 16]),
                                    op=mybir.AluOpType.is_equal)

            with tc.tile_pool(name="sbuf", bufs=1) as pool:
                for t in range(dbg_tiles or T):
                    lo = t * P
                    xt = pool.tile([P, 4], dt.float32)
                    nc.sync.dma_start(out=xt[:], in_=xin[lo:lo + P, :])
                    ib = pool.tile([P, BOX], dt.int32)
                    nc.sync.dma_start(out=ib[:], in_=ibase[lo:lo + P, :])
                    tb = pool.tile([P, 1], dt.int32)
                    nc.sync.dma_start(out=tb[:], in_=tidx[lo:lo + P, :])

                    # per-axis terms, reference-exact f32 order
                    sq = pool.tile([P, 3 * BOX], dt.float32)
                    for ax in range(3):
                        axv = sq[:, ax * BOX:(ax + 1) * BOX]
                        nc.vector.tensor_tensor(
                            out=axv, in0=xt[:, ax:ax + 1].broadcast_to(
                                [P, BOX]),
                            in1=five[:], op=mybir.AluOpType.add)
                        nc.vector.tensor_tensor(
                            out=axv, in0=axv,
                            in1=gc[:, ax:ax + 1].broadcast_to([P, BOX]),
                            op=mybir.AluOpType.mult)
                        nc.vector.tensor_tensor(
                            out=axv, in0=axv, in1=axv,
                            op=mybir.AluOpType.mult)
                    dx2 = sq[:, 0:BOX]
                    dy2 = sq[:, BOX:2 * BOX]
                    dz2 = sq[:, 2 * BOX:3 * BOX]

                    xy = pool.tile([P, BOX * BOX], dt.float32)
                    nc.vector.tensor_tensor(
                        out=xy[:].rearrange("p (b a) -> p b a", b=BOX),
                        in0=dy2.unsqueeze(2).broadcast_to([P, BOX, BOX]),
                        in1=dx2.unsqueeze(1).broadcast_to([P, BOX, BOX]),
                        op=mybir.AluOpType.add)
                    d2 = pool.tile([P, BOX ** 3], dt.float32)
                    nc.vector.tensor_tensor(
                        out=d2[:].rearrange("p (c b a) -> p c b a",
                                            c=BOX, b=BOX),
                        in0=xy[:].rearrange("p (b a) -> p b a", b=BOX)
                            .unsqueeze(1).broadcast_to([P, BOX, BOX, BOX]),
                        in1=dz2.unsqueeze(2).unsqueeze(3)
                            .broadcast_to([P, BOX, BOX, BOX]),
                        op=mybir.AluOpType.add)

                    mask = pool.tile([P, BOX ** 3], dt.float32)
                    nc.vector.tensor_scalar(
                        mask[:], d2[:], float(RMAX * RMAX), None,
                        mybir.AluOpType.is_le)

                    # u = sqrt(d2*1e4), clipped to 302
                    u = pool.tile([P, BOX ** 3], dt.float32)
                    nc.scalar.activation(u[:], d2[:],
                                         mybir.ActivationFunctionType.Sqrt,
                                         scale=1.0e4)
                    nc.vector.tensor_scalar(u[:], u[:], 302.0, None,
                                            mybir.AluOpType.min)
                    # floor via the magic-number trick: ilo = RN(u-0.5)
                    # (ties land on exact integers where the lerp is
                    # continuous, so either neighbor is fine)
                    ilof = pool.tile([P, BOX ** 3], dt.float32)
                    nc.vector.tensor_scalar(ilof[:], u[:], 8388607.5,
                                            8388608.0, mybir.AluOpType.add,
                                            mybir.AluOpType.subtract)
                    w = pool.tile([P, BOX ** 3], dt.float32)
                    nc.vector.tensor_tensor(out=w[:], in0=u[:], in1=ilof[:],
                                            op=mybir.AluOpType.subtract)
                    ilo16 = pool.tile([P, BOX ** 3], dt.uint16)
                    nc.vector.tensor_copy(ilo16[:], ilof[:])
                    ihi16 = pool.tile([P, BOX ** 3], dt.uint16)
                    nc.vector.tensor_scalar(ihi16[:], ilo16[:], 1, None,
                                            mybir.AluOpType.add)

                    # per-atom table into SBUF: one offset per partition,
                    # 304 contiguous bytes from tblu[tidx[p]:...]
                    tsb = pool.tile([P, NRD2], dt.uint8)
                    nc.gpsimd.indirect_dma_start(
                        out=tsb[:], out_offset=None,
                        in_=tblq_flat,
                        in_offset=IndirectOffsetOnAxis(ap=tb[:, 0:1], axis=0))

                    # per-point lookups via indirect_copy: each 16-partition
                    # group shares the wrapped index list, which in point-
                    # major order is exactly ilo16/ihi16 in place; each
                    # partition's own values sit at lane j == p%16 and are
                    # extracted with a select-mask multiply + X-reduce.
                    lov = pool.tile([P, BOX ** 3], dt.float32)
                    hiv = pool.tile([P, BOX ** 3], dt.float32)
                    HNP = 64
                    gth = pool.tile([P, 16 * HNP], dt.uint8)
                    tmpg = pool.tile([P, 16 * HNP], dt.float32)
                    for half in range(BOX ** 3 // HNP):
                        fs = slice(half * HNP, (half + 1) * HNP)
                        for isrc, dstv in ((ilo16, lov), (ihi16, hiv)):
                            nc.gpsimd.indirect_copy(
                                out=gth[:], data=tsb[:],
                                idxs=isrc[:, fs],
                                i_know_ap_gather_is_preferred=True)
                            nc.vector.tensor_tensor(
                                out=tmpg[:].rearrange("p (f j) -> p f j",
                                                      j=16),
                                in0=gth[:].rearrange("p (f j) -> p f j",
                                                     j=16),
                                in1=sel[:].unsqueeze(1).broadcast_to(
                                    [P, HNP, 16]),
                                op=mybir.AluOpType.mult)
                            nc.vector.tensor_reduce(
                                out=dstv[:, fs],
                                in_=tmpg[:].rearrange("p (f j) -> p f j",
                                                      j=16),
                                axis=mybir.AxisListType.X,
                                op=mybir.AluOpType.add)

                    val = pool.tile([P, BOX ** 3], dt.float32)
                    nc.vector.tensor_tensor(out=val[:], in0=hiv[:],
                                            in1=lov[:],
                                            op=mybir.AluOpType.subtract)
                    nc.vector.tensor_tensor(out=val[:], in0=val[:], in1=w[:],
                                            op=mybir.AluOpType.mult)
                    nc.vector.tensor_tensor(out=val[:], in0=val[:],
                                            in1=lov[:],
                                            op=mybir.AluOpType.add)
                    nc.vector.tensor_scalar(val[:], val[:], xt[:, 3:4], None,
                                            mybir.AluOpType.mult)
                    nc.vector.tensor_tensor(out=val[:], in0=val[:],
                                            in1=mask[:],
                                            op=mybir.AluOpType.mult)

                    # scatter one contiguous SPAN (11*140+12 elems) per atom
                    # per dc: indirect scatter consumes ONE index per
                    # partition and moves that partition's contiguous run.
                    # Row gaps carry zeros (accumulate-add of 0).
                    for dc in range(0 if dbg_noscatter else BOX):
                        vp = pool.tile([P, SPAN], dt.float32)
                        nc.vector.memset(vp[:], 0.0)
                        v3 = val[:].rearrange("p (c b a) -> p c b a",
                                              c=BOX, b=BOX)
                        nc.vector.tensor_copy(
                            vp[:, 0:11 * HB].rearrange("p (b a) -> p b a",
                                                       a=HB)[:, :, 0:BOX],
                            v3[:, dc, 0:11, :])
                        nc.vector.tensor_copy(vp[:, 11 * HB:SPAN],
                                              v3[:, dc, 11, :])
                        if dbg_dump and t == 0 and dc == 0:
                            nc.sync.dma_start(out=dval[:], in_=val[:])
                            nc.sync.dma_start(out=dvp[:], in_=vp[:])
                        gsel = grids[(t * BOX + dc) % NGRID]
                        nc.gpsimd.indirect_dma_start(
                            out=gsel[:].unsqueeze(1),
                            out_offset=IndirectOffsetOnAxis(
                                ap=ib[:, dc:dc + 1], axis=0),
                            in_=vp[:], in_offset=None,
                            compute_op=mybir.AluOpType.add)

            # ---- epilogue: combine grids in SBUF, fold halos, quantize ----
            with tc.tile_pool(name="epi", bufs=1) as epi:
                for h in range(0 if dbg_noepi else 2):
                    c0p = h * (SLAB // 2)
                    sl = epi.tile([SLAB // 2, PLANE], dt.float32)
                    slb = epi.tile([SLAB // 2, PLANE], dt.float32)
                    nc.sync.dma_start(
                        out=sl[:],
                        in_=grids[0][c0p * PLANE:(c0p + SLAB // 2) * PLANE]
                            .rearrange("(p f) -> p f", p=SLAB // 2))
                    nc.sync.dma_start(
                        out=slb[:],
                        in_=grids[1][c0p * PLANE:(c0p + SLAB // 2) * PLANE]
                            .rearrange("(p f) -> p f", p=SLAB // 2))
                    nc.vector.tensor_tensor(out=sl[:], in0=sl[:], in1=slb[:],
                                            op=mybir.AluOpType.add)
                    s3 = sl[:].rearrange("p (b a) -> p b a", b=HB)
                    # a-fold then b-fold
                    nc.vector.tensor_tensor(
                        out=s3[:, :, 0:BOX], in0=s3[:, :, 0:BOX],
                        in1=s3[:, :, G:HB], op=mybir.AluOpType.add)
                    nc.vector.tensor_tensor(
                        out=s3[:, 0:BOX, 0:G], in0=s3[:, 0:BOX, 0:G],
                        in1=s3[:, G:HB, 0:G], op=mybir.AluOpType.add)
                    m16 = epi.tile([SLAB // 2, 1], dt.float32)
                    nc.vector.tensor_reduce(
                        out=m16[:], in_=s3[:, 0:G, 0:G],
                        axis=mybir.AxisListType.XY, op=mybir.AluOpType.max)
                    nc.vector.tensor_scalar(m16[:], m16[:], 1.0e-37, None,
                                            mybir.AluOpType.max)
                    sc16 = epi.tile([SLAB // 2, 1], dt.float32)
                    nc.vector.reciprocal(sc16[:], m16[:])
                    nc.vector.tensor_scalar(sc16[:], sc16[:], OUTSCALE, None,
                                            mybir.AluOpType.mult)
                    # reuse slb's buffer for the scaled slab
                    qf = slb[:, 0:G * G]
                    nc.vector.tensor_tensor(
                        out=qf.rearrange("p (b a) -> p b a", b=G),
                        in0=s3[:, 0:G, 0:G],
                        in1=sc16[:].unsqueeze(2).broadcast_to(
                            [SLAB // 2, G, G]),
                        op=mybir.AluOpType.mult)
                    q8t = epi.tile([SLAB // 2, G * G], dt.uint8)
                    nc.vector.tensor_copy(q8t[:], qf)
                    nc.sync.dma_start(
                        out=vout[c0p:c0p + SLAB // 2, :], in_=q8t[:])
                    nc.sync.dma_start(
                        out=vmax[c0p:c0p + SLAB // 2, :], in_=m16[:])
                if dbg_noepi:
                    zz8 = epi.tile([SLAB, G * G], dt.uint8)
                    nc.vector.memset(zz8[:], 0)
                    zzf = epi.tile([SLAB, 1], dt.float32)
                    nc.vector.memset(zzf[:], 1.0)
                    nc.sync.dma_start(out=vout[:], in_=zz8[:])
                    nc.sync.dma_start(out=vmax[:], in_=zzf[:])
    return nc


def _split_multi_waits(nc, max_waits=1):
    """The walrus build in this env rejects >1 sync-wait per instruction.
    Keep one wait on the original instruction and hoist extras onto
    single-wait NoOps inserted just before it on the same engine."""
    import concourse.mybir as mybir

    n_new = 0
    for fn in nc.m.functions:
        for bb in fn.blocks:
            insts = list(bb.instructions)
            out = []
            changed = False
            for inst in insts:
                si = getattr(inst, "sync_info", None)
                w = list(si.on_wait) if si is not None and si.on_wait else []
                if len(w) > max_waits:
                    head, tail = w[:-max_waits], w[-max_waits:]
                    for j, cond in enumerate(head):
                        nop = mybir.InstNoOp(
                            name=f"{inst.name}-w{j}", ins=[], outs=[])
                        nop.engine = inst.engine
                        nop.sync_info = mybir.SyncInfo(
                            on_wait=[cond], on_update=[])
                        out.append(nop)
                        n_new += 1
                    si.on_wait = tail
                    inst.sync_info = si
                    changed = True
                out.append(inst)
            if changed:
                bb.instructions = out
    return n_new


# ------------------------------------------------------------- spmd runner

def _spmd_runner(nc, n_cores=8):
    """Build (once) a cached jitted shard_map executable for `nc`."""
    key = id(nc)
    if key in _JIT_CACHE:
        return _JIT_CACHE[key]
    import jax
    import numpy as _np
    from jax.sharding import Mesh, PartitionSpec
    from jax.experimental.shard_map import shard_map
    import concourse.mybir as mybir
    from concourse import bass2jax as b2j

    b2j.install_neuronx_cc_hook()
    partition_name = (nc.partition_id_tensor.name
                      if nc.partition_id_tensor else None)
    in_names, out_names, out_avals, zero_shapes = [], [], [], []
    for alloc in nc.m.functions[0].allocations:
        if not isinstance(alloc, mybir.MemoryLocationSet):
            continue
        name = alloc.memorylocations[0].name
        if alloc.kind == "ExternalInput":
            if name != partition_name:
                in_names.append(name)
        elif alloc.kind == "ExternalOutput":
            shape = tuple(alloc.tensor_shape)
            dtype = mybir.dt.np(alloc.dtype)
            out_avals.append(jax.core.ShapedArray(shape, dtype))
            out_names.append(name)
            zero_shapes.append((shape, dtype))
    n_params = len(in_names)
    all_names = list(in_names) + list(out_names)
    if partition_name is not None:
        all_names.append(partition_name)
    donate = tuple(range(n_params, n_params + len(out_names)))

    def _body(*args):
        operands = list(args)
        if partition_name is not None:
            operands.append(b2j.partition_id_tensor())
        return tuple(b2j._bass_exec_p.bind(
            *operands, out_avals=tuple(out_avals), in_names=tuple(all_names),
            out_names=tuple(out_names), lowering_input_output_aliases=(),
            sim_require_finite=True, sim_require_nnan=True, nc=nc))

    devices = jax.devices()[:n_cores]
    mesh = Mesh(_np.asarray(devices), ("core",))
    nio = n_params + len(out_names)
    sharded = jax.jit(
        shard_map(_body, mesh=mesh, in_specs=(PartitionSpec("core"),) * nio,
                  out_specs=(PartitionSpec("core"),) * len(out_names),
                  check_rep=False),
        donate_argnums=donate, keep_unused=True)

    def run(in_maps):
        import jax as _jax
        concat_in = [
            _np.concatenate([_np.asarray(m[name]) for m in in_maps], axis=0)
            for name in in_names]
        concat_zeros = [
            _np.zeros((n_cores * s[0], *s[1:]), d) for (s, d) in zero_shapes]
        outs = sharded(*concat_in, *concat_zeros)
        outs = _jax.block_until_ready(outs)
        return [
            {name: _np.asarray(outs[i]).reshape(n_cores, *out_avals[i].shape)[c]
             for i, name in enumerate(out_names)}
            for c in range(n_cores)]

    _JIT_CACHE[key] = run
    return run


# ----------------------------------------------------------- host reference

def _emulate_core(pk, g2c):
    """Numpy replica of the device program for one core's packed inputs."""
    xinp = pk["xin"]
    ibase = pk["ibase"]
    tblq = pk["tblu"][pk["tidx"][:, 0] // NRD2]
    slots = xinp.shape[0]
    g00, g11, g22 = f32(g2c[0, 0]), f32(g2c[1, 1]), f32(g2c[2, 2])
    k = np.arange(BOX, dtype=f32)
    dxv = (xinp[:, 0:1] + (f32(5.0) - k)[None, :]).astype(f32)
    dyv = (xinp[:, 1:2] + (f32(5.0) - k)[None, :]).astype(f32)
    dzv = (xinp[:, 2:3] + (f32(5.0) - k)[None, :]).astype(f32)
    dx2 = (g00 * dxv).astype(f32) ** 2
    dy2 = (g11 * dyv).astype(f32) ** 2
    dz2 = (g22 * dzv).astype(f32) ** 2
    xy = (dx2[:, None, :] + dy2[:, :, None]).astype(f32)
    d2 = (xy[:, None, :, :] + dz2[:, :, None, None]).astype(f32)
    mask = (d2 <= f32(RMAX * RMAX)).astype(f32)
    u = np.sqrt((d2 * f32(1.0e4)).astype(f32)).astype(f32)
    uc = np.minimum(u, f32(302.0))
    w = np.mod(uc, f32(1.0)).astype(f32)
    ilo = (uc - w).astype(np.int32).reshape(slots, -1)
    lo = np.take_along_axis(tblq, ilo, axis=1).astype(f32)
    hi = np.take_along_axis(tblq, ilo + 1, axis=1).astype(f32)
    wf = w.reshape(slots, -1)
    val = ((lo + wf * (hi - lo)) * xinp[:, 3:4]).astype(f32)
    val = (val * mask.reshape(slots, -1)).astype(f32)

    grid = np.zeros(GRID_PAD, np.float64)
    db = np.arange(BOX)
    da = np.arange(BOX)
    idx = (ibase[:, :, None, None].astype(np.int64)
           + db[None, None, :, None] * HB + da[None, None, None, :])
    np.add.at(grid, idx.reshape(-1), val.reshape(-1).astype(np.float64))
    g3 = grid[:GRID_CORE].reshape(SLAB, HB, HB)
    g3[:, :, 0:BOX] += g3[:, :, G:HB]
    g3[:, 0:BOX, :G] += g3[:, G:HB, :G]
    slab = g3[:, :G, :G].astype(f32)
    m32 = np.maximum(slab.max(axis=(1, 2)), 1e-37).astype(f32)
    q = np.round(slab * (f32(OUTSCALE) / m32[:, None, None])).astype(np.uint8)
    return {"vout": q.reshape(SLAB, G * G), "vmax": m32.reshape(SLAB, 1)}


def _reference_numpy(coordinates, active, occupancies, lmax,
                     radial_densities, grid_to_cartesian):
    """Slow but fully general fallback (port of the reference)."""
    L = int(np.floor(2 * float(np.max(lmax)))) + 1
    B, N = coordinates.shape[:2]
    rmax2 = f32(RMAX * RMAX)
    ca, cb, cc = (coordinates[..., 0], coordinates[..., 1],
                  coordinates[..., 2])
    amin = np.ceil(ca - lmax[0]); amax = np.floor(ca + lmax[0])
    bmin = np.ceil(cb - lmax[1]); bmax = np.floor(cb + lmax[1])
    cmin = np.ceil(cc - lmax[2]); cmax = np.floor(cc + lmax[2])
    oc, ob, oa = np.meshgrid(np.arange(L, dtype=f32),
                             np.arange(L, dtype=f32),
                             np.arange(L, dtype=f32), indexing="ij")
    oc, ob, oa = oc.reshape(-1), ob.reshape(-1), oa.reshape(-1)
    out = np.zeros((B, G * G * G), dtype=f32)
    g = grid_to_cartesian
    max_idx = radial_densities.shape[2] - 1
    for b in range(B):
        grid_c = cmin[b][:, None] + oc
        grid_b = bmin[b][:, None] + ob
        grid_a = amin[b][:, None] + oa
        inbox = ((grid_c <= cmax[b][:, None]) & (grid_b <= bmax[b][:, None])
                 & (grid_a <= amax[b][:, None]))
        dcv = cc[b][:, None] - grid_c
        dbv = cb[b][:, None] - grid_b
        dav = ca[b][:, None] - grid_a
        dz = g[2, 2] * dcv
        dy = g[1, 2] * dcv + g[1, 1] * dbv
        dx = g[0, 2] * dcv + g[0, 1] * dbv + g[0, 0] * dav
        d2 = dx * dx + dy * dy + dz * dz
        mask = inbox & (d2 <= rmax2) & active[b][:, None]
        r = np.sqrt(np.where(mask, d2, f32(1.0)).astype(f32))
        rad_c = r / f32(RSTEP)
        ilo = np.clip(np.floor(rad_c).astype(np.int32), 0, max_idx)
        ihi = np.clip(ilo + 1, 0, max_idx)
        w_hi = rad_c - ilo.astype(f32)
        w_lo = f32(1.0) - w_hi
        d_lo = np.take_along_axis(radial_densities[b], ilo, axis=1)
        d_hi = np.take_along_axis(radial_densities[b], ihi, axis=1)
        vals = occupancies[b][:, None] * (w_lo * d_lo + w_hi * d_hi)
        vals = np.where(mask, vals, f32(0.0)).astype(f32)
        ci = np.remainder(grid_c, G).astype(np.int64)
        bi = np.remainder(grid_b, G).astype(np.int64)
        ai = np.remainder(grid_a, G).astype(np.int64)
        flat = ((ci * G + bi) * G + ai).reshape(-1)
        out[b] = np.bincount(flat, weights=vals.reshape(-1).astype(np.float64),
                             minlength=G ** 3).astype(f32)
    return out.reshape(B, G, G, G)


# ------------------------------------------------------------------- kernel

def kernel(coordinates, active, occupancies, lmax, radial_densities,
           grid_to_cartesian):
    global LAST_EXEC_NS, LAST_PROFILE
    coords = np.asarray(coordinates, f32)
    occ = np.asarray(occupancies, f32)
    act = np.asarray(active)
    tbl = np.asarray(radial_densities, f32)
    g2c = np.asarray(grid_to_cartesian, f32)
    lmaxa = np.asarray(lmax, f32)
    B, N, _ = coords.shape

    general = (not np.allclose(np.asarray(g2c, np.float64),
                               np.diag(np.diag(np.asarray(g2c, np.float64))),
                               atol=1e-12)
               or not np.allclose(lmaxa, 6.0)
               or tbl.shape[2] != NRAD or B != 2
               or coords.min() < 0 or coords.max() >= G)
    if general:
        return _reference_numpy(coords, act, occ, lmaxa, tbl, g2c)

    cores = []
    for b in range(B):
        for q in range(4):
            cores.append((b, q, _prep_core(coords[b], occ[b], act[b],
                                           tbl[b], q)))
    T = max((c["slots"] + P - 1) // P for _, _, c in cores)
    NU = max(1536, max(c["n"] for _, _, c in cores))
    packed = [_pack_core(c, T, NU) for _, _, c in cores]
    gcol = np.zeros((P, 4), f32)
    gcol[:, 0] = g2c[0, 0]
    gcol[:, 1] = g2c[1, 1]
    gcol[:, 2] = g2c[2, 2]
    ins = [{"xin": pk["xin"], "ibase": pk["ibase"], "tidx": pk["tidx"],
            "tblu": pk["tblu"], "gcst": gcol} for pk in packed]

    results = None
    if os.environ.get("KERNEL_FORCE_HOST", "0") != "1":
        try:
            if (T, NU) not in _PROG_CACHE:
                prog = _build_program(T, NU)
                _split_multi_waits(prog)
                _PROG_CACHE[(T, NU)] = prog
            import time as _time
            runner = _spmd_runner(_PROG_CACHE[(T, NU)])
            results = runner(ins)
            if os.environ.get("KERNEL_TRACE", "0") == "1":
                best = None
                try:
                    for _ in range(4):
                        t0 = _time.perf_counter()
                        runner(ins)
                        dt = _time.perf_counter() - t0
                        best = dt if best is None else min(best, dt)
                except Exception as te:
                    print(f"[kernel] warm timing stopped: "
                          f"{type(te).__name__}", file=sys.stderr)
                LAST_EXEC_NS = int(best * 1e9) if best is not None else None
        except Exception as e:  # pragma: no cover
            print(f"[kernel] device path failed ({type(e).__name__}: {e}); "
                  f"host fallback", file=sys.stderr)
            results = None
    if results is None:
        results = [_emulate_core(pk, g2c) for pk in packed]

    out = np.zeros((B, G, G, G), f32)
    for (b, q, _), r in zip(cores, results):
        q8 = np.asarray(r["vout"]).reshape(SLAB, G, G).astype(f32)
        m32 = np.asarray(r["vmax"]).reshape(SLAB, 1, 1)
        out[b, SLAB * q:SLAB * (q + 1)] = q8 * (m32 / f32(OUTSCALE))

    # all-integer-coordinate correction (reference box starts one earlier)
    isint = (coords == np.floor(coords)).all(axis=-1) & act
    for b, n in zip(*np.nonzero(isint)):
        ca, cb, cc = (int(coords[b, n, 0]), int(coords[b, n, 1]),
                      int(coords[b, n, 2]))
        v = occ[b, n] * tbl[b, n, NRAD - 2]
        out[b, (cc - 6) % G, cb % G, ca % G] += v
        out[b, cc % G, (cb - 6) % G, ca % G] += v
        out[b, cc % G, cb % G, (ca - 6) % G] += v
    return out
